# revision 21
# baseline (speedup 1.0000x reference)
"""Trainium2 Bass kernel for nn_G_Tensor3D (embedding_lookup / bilinear grid + MLP).

The reference's query coordinates form a fixed regular lattice: the gather
index/weight for output pixel (i, j) depends only on (i//2, i&1) in y and
(j//2, j&1) in x. Per parity there is one (cell offset, lerp weight) pattern;
offsets land in {0, 1, 2} relative to r=i//2 / k=j//2 (the float->int cast may
truncate OR round-to-nearest depending on backend, so the pattern is derived
from the actual input arrays at run time and verified exactly).

The bilinear interpolation folds into MLP layer 1. Key trick: each column
parity pj has exactly 2 x-taps {ox_pj, ox_pj+1}. Two accumulating matmuls
with rhs column offsets s and s+1 (s = min ox) compute, for ALL four parity
groups at once,

  psum[(g,mf), j] = WA[.,(g,mf)]^T T[., j+s] + WB[.,(g,mf)]^T T[., j+s+1]
                  = h1_pre[g, mf, j - (ox_pj - s)]

i.e. groups whose ox exceeds s come out shifted by one column. Columns never
mix in the remaining per-column MLP (block-diagonal layers 2/3), so the shift
rides through to the output and is undone during host-side assembly; the one
missing image column per shifted parity (x = XD-1) is computed exactly on
host. This cuts layer 1 from 3 full 512-column PE streams to 2.

The full per-core input (64 row-triple tiles, 6.4 MB bf16) is DMA'd into
SBUF up front in 8 chunked transfers with 8320-byte per-partition contiguous
runs (vs 1040B when loading per row-pair), so the compute loop carries no
DMA waits. Per row-pair: 2 accumulating K=96 matmuls (layer 1) -> relu ->
block-diag W2 matmul -> relu -> W3 matmul -> copy out. Matmul operands bf16,
PSUM fp32, output fp32.
"""

import numpy as np

GX = 512      # grid side
NF = 32       # features
XD = 1024     # output image side
NCORES = 8
RPC = 64      # row pairs (output image row pairs) per core
PADX = 520    # padded free dim of a data row window (514 used)
# input DMA chunk sizes in row-pairs. Chunks complete FIFO on
# qSyncDynamicHW but their completion semaphores arrive only ~2.8us apart
# (HBM receipt latency under load), while the PE consumes one row-pair per
# ~0.94us — so chunks must carry >=3 row-pairs each to stay ahead.
# chunk 0 additionally carries the 256 layer-1 weight columns (see WCOLS)
CHUNKS = [4, 4, 8, 8, 8, 8, 8, 8, 8]
assert sum(CHUNKS) == RPC
WCOLS = 256   # wab weight columns prepended to the tripx DRAM tensor

_CACHE = {}


def _build_nc():
    from concourse import bass, mybir
    from concourse import tile

    f32 = mybir.dt.float32
    bf16 = mybir.dt.bfloat16
    Relu = mybir.ActivationFunctionType.Relu
    Ident = mybir.ActivationFunctionType.Identity
    Add = mybir.AluOpType.add
    Max = mybir.AluOpType.max

    nc = bass.Bass()
    # whole per-core input, row-pair-major along the free dim; cols 0:WCOLS
    # hold the layer-1 folded weights (WA | WB) so one DMA delivers both the
    # weights and the first row-pair (one completion latency, not two)
    d_trip = nc.declare_dram_parameter(
        "tripx", [3 * NF, WCOLS + RPC * PADX], bf16, isOutput=False)
    # bd packs block-diag W2 [cols 0:128] and block-diag W3 [cols 128:132]
    d_bd = nc.declare_dram_parameter("bd", [128, 132], bf16, isOutput=False)
    # biases: col 0 = b1 tiled, col 1 = b2 tiled
    d_bias = nc.declare_dram_parameter("bias", [128, 2], f32, isOutput=False)
    # out row p = 32*(rp%4) + g (g = 2*pj+pi, rows 4..31 of each 32-block
    # unused); col = 512*(rp//4) + n
    d_out = nc.declare_dram_parameter("out", [128, (RPC // 4) * 512], f32,
                                      isOutput=True)

    with tile.TileContext(nc) as tc:
        with (
            tc.tile_pool(name="const", bufs=1) as cpool,
            tc.tile_pool(name="hid1", bufs=2) as h1pool,
            tc.tile_pool(name="hid2", bufs=4) as h2pool,
            tc.tile_pool(name="ps1", bufs=2, space="PSUM") as ps1,
            tc.tile_pool(name="ps2", bufs=2, space="PSUM") as ps2,
            tc.tile_pool(name="ps3", bufs=2, space="PSUM") as ps3,
        ):
            tBd = cpool.tile([128, 132], bf16)
            nc.gpsimd.dma_start(tBd[:], d_bd[:])
            tb = cpool.tile([128, 2], f32)
            nc.gpsimd.dma_start(tb[:], d_bias[:])
            # warm ACT/DVE vector clocks on the const-DMA semaphore so the
            # in-loop relu instructions carry a single (PE) sync wait — the
            # walrus AC/DVE instruction structs have very few wait slots
            scr = cpool.tile([128, 2], f32)
            nc.scalar.activation(scr[:, 0:1], tb[:, 0:1], Ident)
            nc.vector.tensor_copy(scr[:, 1:2], tb[:, 1:2])

            # whole input resident in SBUF; chunks complete FIFO so compute
            # on early row-pairs starts while later chunks stream in.
            # chunk 0's tile starts with the wab weight columns
            tin = [cpool.tile(
                [3 * NF, cw * PADX + (WCOLS if c == 0 else 0)], bf16,
                name=f"tin{c}") for c, cw in enumerate(CHUNKS)]
            cbase = [0]
            for cw in CHUNKS:
                cbase.append(cbase[-1] + cw)
            for c, cw in enumerate(CHUNKS):
                lo = 0 if c == 0 else WCOLS + cbase[c] * PADX
                nc.sync.dma_start(
                    tin[c][:], d_trip[:, lo:WCOLS + cbase[c + 1] * PADX])
            tW = tin[0]  # wab lives in cols 0:WCOLS of chunk 0's tile

            # single persistent output buffer: each 512-col slice is written
            # exactly once, so the PSUM->SBUF copies carry no WAR waits
            o_all = cpool.tile([128, (RPC // 4) * 512], f32)

            # warm up the PE's HAM clock-gate during the ~4us chunk-0 DMA
            # wait: ~6 cold matmuls (~3.6us) of garbage data into a PSUM
            # tile that is never read (the first real matmul re-clears it
            # via start=True), so real matmuls begin at 2.4 GHz instead of
            # paying the 1.2 GHz ramp
            wsc = cpool.tile([96, 640], bf16)
            nc.vector.memset(wsc[:], 0.0)
            pwarm = ps1.tile([128, 1024], f32, tag="p1")
            for w in range(6):
                nc.tensor.matmul(pwarm[:, 0:512], wsc[:, 0:128],
                                 wsc[:, 128:640], start=True, stop=True)

            import bisect
            # software-pipelined emission: at stage p, layer 1 runs for
            # pair p, layer 2 for pair p-1, layer 3 for pair p-2. Each
            # cross-engine dependency (relu -> next matmul) then has a full
            # pair (~1.8us) of PE work as slack, so PE semaphore waits are
            # pre-satisfied and the PE never micro-idles (which would make
            # the HAM clock-gate oscillate).
            NP = RPC // 2
            h1s = [None] * NP
            h2s = [None] * (NP * 2)
            p3 = None
            for stage in range(NP + 2):
                if stage < NP:
                    pair = stage
                    # two row-pairs share one 2-bank PSUM tile so the fixed
                    # ACT pipeline cost amortizes over 1024 columns
                    p1 = ps1.tile([128, 1024], f32, tag="p1")
                    h1 = h1pool.tile([128, 1024], bf16, tag="h1")
                    h1s[pair] = h1
                    for e in range(2):
                        rp = 2 * pair + e
                        c = bisect.bisect_right(cbase, rp) - 1
                        T = tin[c]
                        base = (rp - cbase[c]) * PADX + (WCOLS if c == 0
                                                         else 0)
                        nc.tensor.matmul(p1[:, 512 * e:512 * e + 512],
                                         tW[:, 0:128], T[:, base:base + 512],
                                         start=True, stop=False)
                        nc.tensor.matmul(p1[:, 512 * e:512 * e + 512],
                                         tW[:, 128:256],
                                         T[:, base + 1:base + 513],
                                         start=False, stop=True)
                    nc.scalar.activation(h1[:], p1[:], Relu, bias=tb[:, 0:1])

                if 1 <= stage <= NP:
                    pair = stage - 1
                    h1 = h1s[pair]
                    for e in range(2):
                        rp = 2 * pair + e
                        p2 = ps2.tile([128, 512], f32, tag="p2")
                        nc.tensor.matmul(p2[:], tBd[:, 0:128],
                                         h1[:, 512 * e:512 * e + 512],
                                         start=True, stop=True)
                        h2 = h2pool.tile([128, 512], bf16, tag="h2")
                        h2s[rp] = h2
                        nc.vector.tensor_scalar(h2[:], p2[:], tb[:, 1:2],
                                                0.0, Add, Max)

                if stage >= 2:
                    pair = stage - 2
                    # pack 4 row-pairs' [4,512] layer-3 outputs into one
                    # PSUM bank at partition offsets 32*(rp%4) so one copy
                    # drains 4 row-pairs. e=1 first: its relu2 semaphore
                    # count covers e=0's too, so only one PE wait is emitted
                    if pair % 2 == 0:
                        p3 = ps3.tile([128, 512], f32, tag="p3")
                    for e in (1, 0):
                        rp = 2 * pair + e
                        jp = rp % 4
                        # explicit tile_position: auto-derive rejects
                        # base_partition 96 (bass_types.rs quirk)
                        nc.tensor.matmul(p3[32 * jp:32 * jp + 4, :],
                                         tBd[:, 128:132], h2s[rp][:],
                                         start=True, stop=True,
                                         tile_position=(0, 32 * jp))
                    # b3 is added host-side during assembly
                    if pair % 2 == 1:
                        g4 = pair // 2
                        osl = o_all[:, g4 * 512:(g4 + 1) * 512]
                        nc.scalar.activation(osl, p3[:], Ident)
                        # sync-issued: the engine is idle once input chunks
                        # finish, and issue cost stays off the ACT queue
                        if g4 >= 14:
                            nc.sync.dma_start(
                                d_out[:, g4 * 512:(g4 + 1) * 512],
                                o_all[:, g4 * 512:(g4 + 1) * 512])
                        elif g4 % 2 == 1:
                            nc.sync.dma_start(
                                d_out[:, (g4 - 1) * 512:(g4 + 1) * 512],
                                o_all[:, (g4 - 1) * 512:(g4 + 1) * 512])

    _split_multi_waits(nc, mybir)
    return nc


def _split_multi_waits(nc, mybir):
    """walrus codegen on this toolchain rejects instructions carrying more
    than one semaphore wait ("Too many sync wait commands"). Hoist all but
    the last wait of each instruction onto standalone single-wait
    EventSemaphore nops on the same engine, inserted just before it."""
    n = 0
    for fn in nc.m.functions:
        for blk in fn.blocks:
            has_multi = any(
                inst.sync_info is not None and len(inst.sync_info.on_wait) > 1
                for inst in blk.instructions
            )
            if not has_multi:
                continue
            out = []
            for inst in blk.instructions:
                si = inst.sync_info
                if si is not None and len(si.on_wait) > 1:
                    waits = list(si.on_wait)
                    for w in waits[:-1]:
                        n += 1
                        nop = mybir.InstEventSemaphore(
                            name=f"waitsplit-{n}",
                            engine=inst.engine,
                            ins=[],
                            outs=[],
                            sync_info=mybir.SyncInfo(on_wait=[w], on_update=[]),
                        )
                        out.append(nop)
                    inst.sync_info = mybir.SyncInfo(
                        on_wait=waits[-1:], on_update=list(si.on_update))
                out.append(inst)
            try:
                blk.instructions[:] = out
            except TypeError:
                blk.instructions = out


def get_nc():
    if "nc" not in _CACHE:
        _CACHE["nc"] = _build_nc()
    return _CACHE["nc"]


def _derive_axis(idx0, idx1, w):
    """Per-parity (o0, o1, wfrac) pattern for one axis, with exact verification.

    idx0/idx1: int arrays over the axis coordinate (len XD), already clipped to
    [0, GX-1] by the reference. w: lerp fraction array (len XD).
    Model: idx0[c] == min(c//2 + o0[c&1], GX-1), idx1 == min(idx0+1, GX-1),
           w[c] == wf[c&1].
    """
    pats = []
    c = np.arange(XD)
    k = c // 2
    for p in range(2):
        sel = np.nonzero((c & 1) == p)[0][: GX - 4]  # interior samples
        o0s = idx0[sel] - k[sel]
        wfs = np.asarray(w[sel], dtype=np.float64)
        # offsets must be exactly constant; lerp weights may wobble by a few
        # fp32 ulps (linspace rounding) around the parity constant
        if not np.all(o0s == o0s[0]):
            raise ValueError("coords are not a parity lattice")
        if wfs.max() - wfs.min() > 4e-3:
            raise ValueError("lerp weights not parity-constant")
        o0 = int(o0s[0])
        wf = float(np.median(wfs))
        if not (0 <= o0 <= 1):
            raise ValueError(f"unexpected lattice offset {o0}")
        pats.append((o0, o0 + 1, wf))
    # reconstruction check over the full axis (indices exact, weights approx)
    o0f = np.array([pats[pp][0] for pp in range(2)])[c & 1]
    rec0 = np.minimum(k + o0f, GX - 1)
    rec1 = np.minimum(rec0 + 1, GX - 1)
    wrec = np.array([pats[pp][2] for pp in range(2)])[c & 1]
    if not (np.array_equal(idx0, rec0) and np.array_equal(idx1, rec1)
            and np.max(np.abs(np.asarray(w, np.float64) - wrec)) <= 4e-3):
        raise ValueError("lattice reconstruction mismatch")
    return pats


def host_prep(data, W1, b1, W2, b2, W3, b3, x0, y0, x1, y1, lerp_weights):
    """Build per-core input maps (all numpy, host-side).

    Group order along M is g = 2*pj + pi (pj-major) so each column parity
    occupies one contiguous 64-partition half.
    Returns (in_maps, xshift) where xshift[pj] in {0, 1} is the column shift
    of parity pj's device output (undone in assemble()).
    """
    import ml_dtypes
    bf = ml_dtypes.bfloat16

    data = np.asarray(data, dtype=np.float32)
    W1 = np.asarray(W1, dtype=np.float32)
    W2 = np.asarray(W2, dtype=np.float32)
    W3 = np.asarray(W3, dtype=np.float32)
    b1 = np.asarray(b1, dtype=np.float32).reshape(-1)
    b2 = np.asarray(b2, dtype=np.float32).reshape(-1)
    x0 = np.asarray(x0)
    y0 = np.asarray(y0)
    x1 = np.asarray(x1)
    y1 = np.asarray(y1)
    lerp = np.asarray(lerp_weights, dtype=np.float32)

    # axis-separability check + pattern extraction
    # flat n = i*XD + j: x-axis fields depend on j, y-axis fields on i
    xpat = _derive_axis(x0[:XD], x1[:XD], lerp[:XD, 0])
    ypat = _derive_axis(y0[::XD], y1[::XD], lerp[::XD, 1])
    # verify separability exactly (cheap: compare tiled patterns)
    if not (np.array_equal(x0.reshape(XD, XD), np.broadcast_to(x0[:XD], (XD, XD)))
            and np.array_equal(y0.reshape(XD, XD),
                               np.broadcast_to(y0[::XD, None], (XD, XD)))
            and np.array_equal(x1.reshape(XD, XD), np.broadcast_to(x1[:XD], (XD, XD)))
            and np.array_equal(y1.reshape(XD, XD),
                               np.broadcast_to(y1[::XD, None], (XD, XD)))
            and np.array_equal(lerp[:, 0].reshape(XD, XD),
                               np.broadcast_to(lerp[:XD, 0], (XD, XD)))
            and np.array_equal(lerp[:, 1].reshape(XD, XD),
                               np.broadcast_to(lerp[::XD, 1][:, None], (XD, XD)))):
        raise ValueError("coords not axis-separable")

    # y interp weights per parity over row offsets dy in 0..2
    wy = np.zeros((2, 3), dtype=np.float64)
    for p in range(2):
        o0, o1, wf = ypat[p]
        wy[p, o0] += 1.0 - wf
        wy[p, o1] += wf

    # x: two streams at column offsets s, s+1; per parity the 2 taps are
    # {ox, ox+1} with weights (1-f, f); tap ox rides stream 1 (WA), tap ox+1
    # rides stream 2 (WB); parity output shifted by ox - s columns
    oxs = [xpat[p][0] for p in range(2)]
    s_off = min(oxs)
    xshift = [ox - s_off for ox in oxs]
    if not all(sh in (0, 1) for sh in xshift):
        raise ValueError(f"unsupported x tap spread {oxs}")

    # feature-major rows, x-padded with duplicated edge cols (clip semantics)
    data_t = np.ascontiguousarray(data.transpose(0, 2, 1))       # [512, 32, 512]
    dt_pad = np.zeros((GX, NF, PADX), dtype=np.float32)
    dt_pad[:, :, :GX] = data_t
    dt_pad[:, :, GX] = data_t[:, :, GX - 1]
    dt_pad[:, :, GX + 1] = data_t[:, :, GX - 1]

    # row triples with clipped row indices: [512, 96, PADX]
    r = np.arange(GX)
    trip = np.concatenate(
        [dt_pad, dt_pad[np.minimum(r + 1, GX - 1)],
         dt_pad[np.minimum(r + 2, GX - 1)]], axis=1).astype(bf)

    # folded layer-1 weights, group order g = 2*pj + pi
    wab = np.zeros((3 * NF, 256), dtype=np.float64)
    for pj in range(2):
        _, _, fx = xpat[pj]
        cxa, cxb = 1.0 - fx, fx
        for pi in range(2):
            g = 2 * pj + pi
            for dy in range(3):
                if wy[pi, dy] == 0.0:
                    continue
                blk = wy[pi, dy] * W1
                wab[dy * NF:(dy + 1) * NF, g * NF:(g + 1) * NF] += cxa * blk
                wab[dy * NF:(dy + 1) * NF,
                    128 + g * NF:128 + (g + 1) * NF] += cxb * blk

    bd = np.zeros((128, 132), dtype=np.float32)
    for g in range(4):
        bd[g * NF:(g + 1) * NF, g * NF:(g + 1) * NF] = W2
        bd[g * NF:(g + 1) * NF, 128 + g] = W3[:, 0]

    bias = np.zeros((128, 2), dtype=np.float32)
    bias[:, 0] = np.tile(b1, 4)
    bias[:, 1] = np.tile(b2, 4)

    consts = {"bd": bd.astype(bf), "bias": bias}
    wab16 = wab.astype(bf)

    # the rhs for stream 1 starts at column s_off: fold it into the layout by
    # shifting each row-pair window so on-device offsets are always 0 / +1.
    # wab is prepended as the first WCOLS columns of tripx (delivered by the
    # same first DMA chunk as row-pair 0)
    in_maps = []
    for c in range(NCORES):
        sl = trip[c * RPC:(c + 1) * RPC]                 # [RPC, 96, PADX]
        if s_off:
            sl = np.concatenate(
                [sl[:, :, s_off:], sl[:, :, :s_off]], axis=2)
        m = dict(consts)
        m["tripx"] = np.ascontiguousarray(np.concatenate(
            [wab16, sl.transpose(1, 0, 2).reshape(3 * NF, RPC * PADX)],
            axis=1))
        in_maps.append(m)
    return in_maps, xshift


def _host_column(inp_col, data, W1, b1, W2, b2, W3, b3):
    """Exact (fp32 numpy) MLP for one set of query points.

    inp_col: dict with x0/y0/x1/y1 (int arrays [n]) and w0/w1 ([n,1] fp32).
    Returns [n] fp32 outputs (including b3).
    """
    Ia = data[inp_col["y0"], inp_col["x0"]]
    Ib = data[inp_col["y0"], inp_col["x1"]]
    Ic = data[inp_col["y1"], inp_col["x0"]]
    Id = data[inp_col["y1"], inp_col["x1"]]
    w0, w1 = inp_col["w0"], inp_col["w1"]
    feat = (Ia * (1.0 - w0) * (1.0 - w1) + Ib * w0 * (1.0 - w1)
            + Ic * (1.0 - w0) * w1 + Id * w0 * w1)
    h = np.maximum(feat @ W1 + b1, 0.0)
    h = np.maximum(h @ W2 + b2, 0.0)
    return (h @ W3)[:, 0] + b3


def assemble(results, batch, xshift, host_cols):
    """results: list of 8 dicts with 'out' [4, RPC*512] -> [b, 1, XD, XD].

    Device row order is g = 2*pj + pi. Parity pj's device columns are shifted
    by xshift[pj]: device column k holds the value for x-cell k - xshift[pj].
    host_cols maps image column -> exact host-computed [XD] values (already
    including b3) for the columns the device does not produce.
    """
    b3v, host_vals = host_cols
    blocks = []
    for c in range(NCORES):
        a = np.asarray(results[c]["out"], dtype=np.float32)      # [128, 16*512]
        # row p = 32*jp + g (g = 2*pj + pi), col = 512*g4 + n, rp = 4*g4 + jp
        a = a.reshape(4, 32, RPC // 4, 512)[:, :4]               # [jp, g, g4, n]
        a = a.transpose(2, 0, 1, 3).reshape(RPC, 2, 2, 512)      # [rp, pj, pi, n]
        a = a.transpose(0, 2, 3, 1)                              # [rp, pi, k, pj]
        blocks.append(a.reshape(2 * RPC, XD))
    img = np.concatenate(blocks, axis=0) + b3v                   # [XD, XD]
    out = np.empty((XD, XD), dtype=np.float32)
    for pj in range(2):
        sh = xshift[pj]
        if sh == 0:
            out[:, pj::2] = img[:, pj::2]
        else:
            # x-cell m (image col 2m+pj) is at device col m+sh
            out[:, pj:XD - 2 * sh:2] = img[:, pj + 2 * sh::2]
    for col, vals in host_vals.items():
        out[:, col] = vals
    return np.broadcast_to(out, (batch, 1, XD, XD)).copy()


def run_device(in_maps, trace=False, **kw):
    try:
        from concourse.bass_utils import run_bass_kernel_spmd
    except ImportError:
        import sys
        sys.path.insert(0, "/opt/trn_rl_repo")
        from concourse.bass_utils import run_bass_kernel_spmd
    nc = get_nc()
    return run_bass_kernel_spmd(nc, in_maps, list(range(NCORES)), trace=trace, **kw)


def postprocess(results, batch, xshift, data, W1, b1, W2, b2, W3, b3,
                x0, y0, x1, y1, lerp_weights):
    """Exact host values for device-undefined columns + final assembly."""
    data32 = np.asarray(data, dtype=np.float32)
    W1a = np.asarray(W1, np.float32)
    W2a = np.asarray(W2, np.float32)
    W3a = np.asarray(W3, np.float32)
    b1a = np.asarray(b1, np.float32).reshape(-1)
    b2a = np.asarray(b2, np.float32).reshape(-1)
    b3v = np.float32(np.asarray(b3).reshape(-1)[0])
    x0a = np.asarray(x0)
    y0a = np.asarray(y0)
    x1a = np.asarray(x1)
    y1a = np.asarray(y1)
    lerp = np.asarray(lerp_weights, dtype=np.float32)
    host_vals = {}
    for pj, sh in enumerate(xshift):
        for m in range(512 - sh, 512):
            col = 2 * m + pj
            n = np.arange(XD) * XD + col
            ic = {"x0": x0a[n], "y0": y0a[n], "x1": x1a[n], "y1": y1a[n],
                  "w0": lerp[n, 0:1], "w1": lerp[n, 1:2]}
            host_vals[col] = _host_column(ic, data32, W1a, b1a, W2a, b2a,
                                          W3a, b3v)
    return assemble(results, batch, xshift, (b3v, host_vals))


def kernel(z, data, W1, b1, W2, b2, W3, b3, x0, y0, x1, y1, lerp_weights,
           **_unused):
    in_maps, xshift = host_prep(data, W1, b1, W2, b2, W3, b3,
                                x0, y0, x1, y1, lerp_weights)
    res = run_device(in_maps)
    batch = np.asarray(z).shape[0]
    return postprocess(res.results, batch, xshift, data, W1, b1, W2, b2,
                       W3, b3, x0, y0, x1, y1, lerp_weights)


# revision 23
# speedup vs baseline: 1.0263x; 1.0263x over previous
"""Trainium2 Bass kernel for nn_G_Tensor3D (embedding_lookup / bilinear grid + MLP).

The reference's query coordinates form a fixed regular lattice: the gather
index/weight for output pixel (i, j) depends only on (i//2, i&1) in y and
(j//2, j&1) in x. Per parity there is one (cell offset, lerp weight) pattern;
offsets land in {0, 1, 2} relative to r=i//2 / k=j//2 (the float->int cast may
truncate OR round-to-nearest depending on backend, so the pattern is derived
from the actual input arrays at run time and verified exactly).

The bilinear interpolation folds into MLP layer 1. Key trick: each column
parity pj has exactly 2 x-taps {ox_pj, ox_pj+1}. Two accumulating matmuls
with rhs column offsets s and s+1 (s = min ox) compute, for ALL four parity
groups at once,

  psum[(g,mf), j] = WA[.,(g,mf)]^T T[., j+s] + WB[.,(g,mf)]^T T[., j+s+1]
                  = h1_pre[g, mf, j - (ox_pj - s)]

i.e. groups whose ox exceeds s come out shifted by one column. Columns never
mix in the remaining per-column MLP (block-diagonal layers 2/3), so the shift
rides through to the output and is undone during host-side assembly; the one
missing image column per shifted parity (x = XD-1) is computed exactly on
host. This cuts layer 1 from 3 full 512-column PE streams to 2.

The full per-core input (64 row-triple tiles, 6.4 MB bf16) is DMA'd into
SBUF up front in 8 chunked transfers with 8320-byte per-partition contiguous
runs (vs 1040B when loading per row-pair), so the compute loop carries no
DMA waits. Per row-pair: 2 accumulating K=96 matmuls (layer 1) -> relu ->
block-diag W2 matmul -> relu -> W3 matmul -> copy out. Matmul operands bf16,
PSUM fp32, output fp32.
"""

import numpy as np

GX = 512      # grid side
NF = 32       # features
XD = 1024     # output image side
NCORES = 8
RPC = 64      # row pairs (output image row pairs) per core
PADX = 520    # padded free dim of a data row window (514 used)
# input DMA chunk sizes in row-pairs: small first chunks so the first
# matmul's data lands early (chunks complete FIFO on qSyncDynamicHW).
# chunk 0 additionally carries the 256 layer-1 weight columns (see WCOLS)
CHUNKS = [1, 1, 2, 4, 8, 8, 8, 8, 8, 8, 8]
assert sum(CHUNKS) == RPC
WCOLS = 256   # wab weight columns prepended to the tripx DRAM tensor

_CACHE = {}


def _build_nc():
    from concourse import bass, mybir
    from concourse import tile

    f32 = mybir.dt.float32
    bf16 = mybir.dt.bfloat16
    Relu = mybir.ActivationFunctionType.Relu
    Ident = mybir.ActivationFunctionType.Identity
    Add = mybir.AluOpType.add
    Max = mybir.AluOpType.max

    nc = bass.Bass()
    # whole per-core input, row-pair-major along the free dim; cols 0:WCOLS
    # hold the layer-1 folded weights (WA | WB) so one DMA delivers both the
    # weights and the first row-pair (one completion latency, not two)
    d_trip = nc.declare_dram_parameter(
        "tripx", [3 * NF, WCOLS + RPC * PADX], bf16, isOutput=False)
    # bd packs block-diag W2 [cols 0:128] and block-diag W3 [cols 128:132]
    d_bd = nc.declare_dram_parameter("bd", [128, 132], bf16, isOutput=False)
    # biases: col 0 = b1 tiled, col 1 = b2 tiled
    d_bias = nc.declare_dram_parameter("bias", [128, 2], f32, isOutput=False)
    # out row p = 32*(rp%4) + g (g = 2*pj+pi, rows 4..31 of each 32-block
    # unused); col = 512*(rp//4) + n
    d_out = nc.declare_dram_parameter("out", [128, (RPC // 4) * 512], f32,
                                      isOutput=True)

    with tile.TileContext(nc) as tc:
        with (
            tc.tile_pool(name="const", bufs=1) as cpool,
            tc.tile_pool(name="hid1", bufs=2) as h1pool,
            tc.tile_pool(name="hid2", bufs=4) as h2pool,
            tc.tile_pool(name="ps1", bufs=2, space="PSUM") as ps1,
            tc.tile_pool(name="ps2", bufs=2, space="PSUM") as ps2,
            tc.tile_pool(name="ps3", bufs=2, space="PSUM") as ps3,
        ):
            tBd = cpool.tile([128, 132], bf16)
            nc.gpsimd.dma_start(tBd[:], d_bd[:])
            tb = cpool.tile([128, 2], f32)
            nc.gpsimd.dma_start(tb[:], d_bias[:])
            # warm ACT/DVE vector clocks on the const-DMA semaphore so the
            # in-loop relu instructions carry a single (PE) sync wait — the
            # walrus AC/DVE instruction structs have very few wait slots
            scr = cpool.tile([128, 2], f32)
            nc.scalar.activation(scr[:, 0:1], tb[:, 0:1], Ident)
            nc.vector.tensor_copy(scr[:, 1:2], tb[:, 1:2])

            # whole input resident in SBUF; chunks complete FIFO so compute
            # on early row-pairs starts while later chunks stream in.
            # chunk 0's tile starts with the wab weight columns
            tin = [cpool.tile(
                [3 * NF, cw * PADX + (WCOLS if c == 0 else 0)], bf16,
                name=f"tin{c}") for c, cw in enumerate(CHUNKS)]
            cbase = [0]
            for cw in CHUNKS:
                cbase.append(cbase[-1] + cw)
            for c, cw in enumerate(CHUNKS):
                lo = 0 if c == 0 else WCOLS + cbase[c] * PADX
                nc.sync.dma_start(
                    tin[c][:], d_trip[:, lo:WCOLS + cbase[c + 1] * PADX])
            tW = tin[0]  # wab lives in cols 0:WCOLS of chunk 0's tile

            # single persistent output buffer: each 512-col slice is written
            # exactly once, so the PSUM->SBUF copies carry no WAR waits
            o_all = cpool.tile([128, (RPC // 4) * 512], f32)

            import bisect
            # software-pipelined emission: at stage p, layer 1 runs for
            # pair p, layer 2 for pair p-1, layer 3 for pair p-2. Each
            # cross-engine dependency (relu -> next matmul) then has a full
            # pair (~1.8us) of PE work as slack, so PE semaphore waits are
            # pre-satisfied and the PE never micro-idles (which would make
            # the HAM clock-gate oscillate).
            NP = RPC // 2
            h1s = [None] * NP
            h2s = [None] * (NP * 2)
            p3 = None
            for stage in range(NP + 2):
                if stage < NP:
                    pair = stage
                    # two row-pairs share one 2-bank PSUM tile so the fixed
                    # ACT pipeline cost amortizes over 1024 columns
                    p1 = ps1.tile([128, 1024], f32, tag="p1")
                    h1 = h1pool.tile([128, 1024], bf16, tag="h1")
                    h1s[pair] = h1
                    for e in range(2):
                        rp = 2 * pair + e
                        c = bisect.bisect_right(cbase, rp) - 1
                        T = tin[c]
                        base = (rp - cbase[c]) * PADX + (WCOLS if c == 0
                                                         else 0)
                        nc.tensor.matmul(p1[:, 512 * e:512 * e + 512],
                                         tW[:, 0:128], T[:, base:base + 512],
                                         start=True, stop=False)
                        nc.tensor.matmul(p1[:, 512 * e:512 * e + 512],
                                         tW[:, 128:256],
                                         T[:, base + 1:base + 513],
                                         start=False, stop=True)
                    nc.scalar.activation(h1[:], p1[:], Relu, bias=tb[:, 0:1])

                if 1 <= stage <= NP:
                    pair = stage - 1
                    h1 = h1s[pair]
                    for e in range(2):
                        rp = 2 * pair + e
                        p2 = ps2.tile([128, 512], f32, tag="p2")
                        nc.tensor.matmul(p2[:], tBd[:, 0:128],
                                         h1[:, 512 * e:512 * e + 512],
                                         start=True, stop=True)
                        h2 = h2pool.tile([128, 512], bf16, tag="h2")
                        h2s[rp] = h2
                        nc.vector.tensor_scalar(h2[:], p2[:], tb[:, 1:2],
                                                0.0, Add, Max)

                if stage >= 2:
                    pair = stage - 2
                    # pack 4 row-pairs' [4,512] layer-3 outputs into one
                    # PSUM bank at partition offsets 32*(rp%4) so one copy
                    # drains 4 row-pairs. e=1 first: its relu2 semaphore
                    # count covers e=0's too, so only one PE wait is emitted
                    if pair % 2 == 0:
                        p3 = ps3.tile([128, 512], f32, tag="p3")
                    for e in (1, 0):
                        rp = 2 * pair + e
                        jp = rp % 4
                        # explicit tile_position: auto-derive rejects
                        # base_partition 96 (bass_types.rs quirk)
                        nc.tensor.matmul(p3[32 * jp:32 * jp + 4, :],
                                         tBd[:, 128:132], h2s[rp][:],
                                         start=True, stop=True,
                                         tile_position=(0, 32 * jp))
                    # b3 is added host-side during assembly
                    if pair % 2 == 1:
                        g4 = pair // 2
                        osl = o_all[:, g4 * 512:(g4 + 1) * 512]
                        nc.scalar.activation(osl, p3[:], Ident)
                        # sync-issued: the engine is idle once input chunks
                        # finish, and issue cost stays off the ACT queue
                        if g4 >= 14:
                            nc.sync.dma_start(
                                d_out[:, g4 * 512:(g4 + 1) * 512],
                                o_all[:, g4 * 512:(g4 + 1) * 512])
                        elif g4 % 2 == 1:
                            nc.sync.dma_start(
                                d_out[:, (g4 - 1) * 512:(g4 + 1) * 512],
                                o_all[:, (g4 - 1) * 512:(g4 + 1) * 512])

    _split_multi_waits(nc, mybir)
    return nc


def _split_multi_waits(nc, mybir):
    """walrus codegen on this toolchain rejects instructions carrying more
    than one semaphore wait ("Too many sync wait commands"). Hoist all but
    the last wait of each instruction onto standalone single-wait
    EventSemaphore nops on the same engine, inserted just before it."""
    n = 0
    for fn in nc.m.functions:
        for blk in fn.blocks:
            has_multi = any(
                inst.sync_info is not None and len(inst.sync_info.on_wait) > 1
                for inst in blk.instructions
            )
            if not has_multi:
                continue
            out = []
            for inst in blk.instructions:
                si = inst.sync_info
                if si is not None and len(si.on_wait) > 1:
                    waits = list(si.on_wait)
                    for w in waits[:-1]:
                        n += 1
                        nop = mybir.InstEventSemaphore(
                            name=f"waitsplit-{n}",
                            engine=inst.engine,
                            ins=[],
                            outs=[],
                            sync_info=mybir.SyncInfo(on_wait=[w], on_update=[]),
                        )
                        out.append(nop)
                    inst.sync_info = mybir.SyncInfo(
                        on_wait=waits[-1:], on_update=list(si.on_update))
                out.append(inst)
            try:
                blk.instructions[:] = out
            except TypeError:
                blk.instructions = out


def get_nc():
    if "nc" not in _CACHE:
        _CACHE["nc"] = _build_nc()
    return _CACHE["nc"]


def _derive_axis(idx0, idx1, w):
    """Per-parity (o0, o1, wfrac) pattern for one axis, with exact verification.

    idx0/idx1: int arrays over the axis coordinate (len XD), already clipped to
    [0, GX-1] by the reference. w: lerp fraction array (len XD).
    Model: idx0[c] == min(c//2 + o0[c&1], GX-1), idx1 == min(idx0+1, GX-1),
           w[c] == wf[c&1].
    """
    pats = []
    c = np.arange(XD)
    k = c // 2
    for p in range(2):
        sel = np.nonzero((c & 1) == p)[0][: GX - 4]  # interior samples
        o0s = idx0[sel] - k[sel]
        wfs = np.asarray(w[sel], dtype=np.float64)
        # offsets must be exactly constant; lerp weights may wobble by a few
        # fp32 ulps (linspace rounding) around the parity constant
        if not np.all(o0s == o0s[0]):
            raise ValueError("coords are not a parity lattice")
        if wfs.max() - wfs.min() > 4e-3:
            raise ValueError("lerp weights not parity-constant")
        o0 = int(o0s[0])
        wf = float(np.median(wfs))
        if not (0 <= o0 <= 1):
            raise ValueError(f"unexpected lattice offset {o0}")
        pats.append((o0, o0 + 1, wf))
    # reconstruction check over the full axis (indices exact, weights approx)
    o0f = np.array([pats[pp][0] for pp in range(2)])[c & 1]
    rec0 = np.minimum(k + o0f, GX - 1)
    rec1 = np.minimum(rec0 + 1, GX - 1)
    wrec = np.array([pats[pp][2] for pp in range(2)])[c & 1]
    if not (np.array_equal(idx0, rec0) and np.array_equal(idx1, rec1)
            and np.max(np.abs(np.asarray(w, np.float64) - wrec)) <= 4e-3):
        raise ValueError("lattice reconstruction mismatch")
    return pats


def host_prep(data, W1, b1, W2, b2, W3, b3, x0, y0, x1, y1, lerp_weights):
    """Build per-core input maps (all numpy, host-side).

    Group order along M is g = 2*pj + pi (pj-major) so each column parity
    occupies one contiguous 64-partition half.
    Returns (in_maps, xshift) where xshift[pj] in {0, 1} is the column shift
    of parity pj's device output (undone in assemble()).
    """
    import ml_dtypes
    bf = ml_dtypes.bfloat16

    data = np.asarray(data, dtype=np.float32)
    W1 = np.asarray(W1, dtype=np.float32)
    W2 = np.asarray(W2, dtype=np.float32)
    W3 = np.asarray(W3, dtype=np.float32)
    b1 = np.asarray(b1, dtype=np.float32).reshape(-1)
    b2 = np.asarray(b2, dtype=np.float32).reshape(-1)
    x0 = np.asarray(x0)
    y0 = np.asarray(y0)
    x1 = np.asarray(x1)
    y1 = np.asarray(y1)
    lerp = np.asarray(lerp_weights, dtype=np.float32)

    # axis-separability check + pattern extraction
    # flat n = i*XD + j: x-axis fields depend on j, y-axis fields on i
    xpat = _derive_axis(x0[:XD], x1[:XD], lerp[:XD, 0])
    ypat = _derive_axis(y0[::XD], y1[::XD], lerp[::XD, 1])
    # verify separability exactly (cheap: compare tiled patterns)
    if not (np.array_equal(x0.reshape(XD, XD), np.broadcast_to(x0[:XD], (XD, XD)))
            and np.array_equal(y0.reshape(XD, XD),
                               np.broadcast_to(y0[::XD, None], (XD, XD)))
            and np.array_equal(x1.reshape(XD, XD), np.broadcast_to(x1[:XD], (XD, XD)))
            and np.array_equal(y1.reshape(XD, XD),
                               np.broadcast_to(y1[::XD, None], (XD, XD)))
            and np.array_equal(lerp[:, 0].reshape(XD, XD),
                               np.broadcast_to(lerp[:XD, 0], (XD, XD)))
            and np.array_equal(lerp[:, 1].reshape(XD, XD),
                               np.broadcast_to(lerp[::XD, 1][:, None], (XD, XD)))):
        raise ValueError("coords not axis-separable")

    # y interp weights per parity over row offsets dy in 0..2
    wy = np.zeros((2, 3), dtype=np.float64)
    for p in range(2):
        o0, o1, wf = ypat[p]
        wy[p, o0] += 1.0 - wf
        wy[p, o1] += wf

    # x: two streams at column offsets s, s+1; per parity the 2 taps are
    # {ox, ox+1} with weights (1-f, f); tap ox rides stream 1 (WA), tap ox+1
    # rides stream 2 (WB); parity output shifted by ox - s columns
    oxs = [xpat[p][0] for p in range(2)]
    s_off = min(oxs)
    xshift = [ox - s_off for ox in oxs]
    if not all(sh in (0, 1) for sh in xshift):
        raise ValueError(f"unsupported x tap spread {oxs}")

    # feature-major rows, x-padded with duplicated edge cols (clip semantics)
    data_t = np.ascontiguousarray(data.transpose(0, 2, 1))       # [512, 32, 512]
    dt_pad = np.zeros((GX, NF, PADX), dtype=np.float32)
    dt_pad[:, :, :GX] = data_t
    dt_pad[:, :, GX] = data_t[:, :, GX - 1]
    dt_pad[:, :, GX + 1] = data_t[:, :, GX - 1]

    # row triples with clipped row indices: [512, 96, PADX]
    r = np.arange(GX)
    trip = np.concatenate(
        [dt_pad, dt_pad[np.minimum(r + 1, GX - 1)],
         dt_pad[np.minimum(r + 2, GX - 1)]], axis=1).astype(bf)

    # folded layer-1 weights, group order g = 2*pj + pi
    wab = np.zeros((3 * NF, 256), dtype=np.float64)
    for pj in range(2):
        _, _, fx = xpat[pj]
        cxa, cxb = 1.0 - fx, fx
        for pi in range(2):
            g = 2 * pj + pi
            for dy in range(3):
                if wy[pi, dy] == 0.0:
                    continue
                blk = wy[pi, dy] * W1
                wab[dy * NF:(dy + 1) * NF, g * NF:(g + 1) * NF] += cxa * blk
                wab[dy * NF:(dy + 1) * NF,
                    128 + g * NF:128 + (g + 1) * NF] += cxb * blk

    bd = np.zeros((128, 132), dtype=np.float32)
    for g in range(4):
        bd[g * NF:(g + 1) * NF, g * NF:(g + 1) * NF] = W2
        bd[g * NF:(g + 1) * NF, 128 + g] = W3[:, 0]

    bias = np.zeros((128, 2), dtype=np.float32)
    bias[:, 0] = np.tile(b1, 4)
    bias[:, 1] = np.tile(b2, 4)

    consts = {"bd": bd.astype(bf), "bias": bias}
    wab16 = wab.astype(bf)

    # the rhs for stream 1 starts at column s_off: fold it into the layout by
    # shifting each row-pair window so on-device offsets are always 0 / +1.
    # wab is prepended as the first WCOLS columns of tripx (delivered by the
    # same first DMA chunk as row-pair 0)
    in_maps = []
    for c in range(NCORES):
        sl = trip[c * RPC:(c + 1) * RPC]                 # [RPC, 96, PADX]
        if s_off:
            sl = np.concatenate(
                [sl[:, :, s_off:], sl[:, :, :s_off]], axis=2)
        m = dict(consts)
        m["tripx"] = np.ascontiguousarray(np.concatenate(
            [wab16, sl.transpose(1, 0, 2).reshape(3 * NF, RPC * PADX)],
            axis=1))
        in_maps.append(m)
    return in_maps, xshift


def _host_column(inp_col, data, W1, b1, W2, b2, W3, b3):
    """Exact (fp32 numpy) MLP for one set of query points.

    inp_col: dict with x0/y0/x1/y1 (int arrays [n]) and w0/w1 ([n,1] fp32).
    Returns [n] fp32 outputs (including b3).
    """
    Ia = data[inp_col["y0"], inp_col["x0"]]
    Ib = data[inp_col["y0"], inp_col["x1"]]
    Ic = data[inp_col["y1"], inp_col["x0"]]
    Id = data[inp_col["y1"], inp_col["x1"]]
    w0, w1 = inp_col["w0"], inp_col["w1"]
    feat = (Ia * (1.0 - w0) * (1.0 - w1) + Ib * w0 * (1.0 - w1)
            + Ic * (1.0 - w0) * w1 + Id * w0 * w1)
    h = np.maximum(feat @ W1 + b1, 0.0)
    h = np.maximum(h @ W2 + b2, 0.0)
    return (h @ W3)[:, 0] + b3


def assemble(results, batch, xshift, host_cols):
    """results: list of 8 dicts with 'out' [4, RPC*512] -> [b, 1, XD, XD].

    Device row order is g = 2*pj + pi. Parity pj's device columns are shifted
    by xshift[pj]: device column k holds the value for x-cell k - xshift[pj].
    host_cols maps image column -> exact host-computed [XD] values (already
    including b3) for the columns the device does not produce.
    """
    b3v, host_vals = host_cols
    blocks = []
    for c in range(NCORES):
        a = np.asarray(results[c]["out"], dtype=np.float32)      # [128, 16*512]
        # row p = 32*jp + g (g = 2*pj + pi), col = 512*g4 + n, rp = 4*g4 + jp
        a = a.reshape(4, 32, RPC // 4, 512)[:, :4]               # [jp, g, g4, n]
        a = a.transpose(2, 0, 1, 3).reshape(RPC, 2, 2, 512)      # [rp, pj, pi, n]
        a = a.transpose(0, 2, 3, 1)                              # [rp, pi, k, pj]
        blocks.append(a.reshape(2 * RPC, XD))
    img = np.concatenate(blocks, axis=0) + b3v                   # [XD, XD]
    out = np.empty((XD, XD), dtype=np.float32)
    for pj in range(2):
        sh = xshift[pj]
        if sh == 0:
            out[:, pj::2] = img[:, pj::2]
        else:
            # x-cell m (image col 2m+pj) is at device col m+sh
            out[:, pj:XD - 2 * sh:2] = img[:, pj + 2 * sh::2]
    for col, vals in host_vals.items():
        out[:, col] = vals
    return np.broadcast_to(out, (batch, 1, XD, XD)).copy()


def run_device(in_maps, trace=False, **kw):
    try:
        from concourse.bass_utils import run_bass_kernel_spmd
    except ImportError:
        import sys
        sys.path.insert(0, "/opt/trn_rl_repo")
        from concourse.bass_utils import run_bass_kernel_spmd
    nc = get_nc()
    return run_bass_kernel_spmd(nc, in_maps, list(range(NCORES)), trace=trace, **kw)


def postprocess(results, batch, xshift, data, W1, b1, W2, b2, W3, b3,
                x0, y0, x1, y1, lerp_weights):
    """Exact host values for device-undefined columns + final assembly."""
    data32 = np.asarray(data, dtype=np.float32)
    W1a = np.asarray(W1, np.float32)
    W2a = np.asarray(W2, np.float32)
    W3a = np.asarray(W3, np.float32)
    b1a = np.asarray(b1, np.float32).reshape(-1)
    b2a = np.asarray(b2, np.float32).reshape(-1)
    b3v = np.float32(np.asarray(b3).reshape(-1)[0])
    x0a = np.asarray(x0)
    y0a = np.asarray(y0)
    x1a = np.asarray(x1)
    y1a = np.asarray(y1)
    lerp = np.asarray(lerp_weights, dtype=np.float32)
    host_vals = {}
    for pj, sh in enumerate(xshift):
        for m in range(512 - sh, 512):
            col = 2 * m + pj
            n = np.arange(XD) * XD + col
            ic = {"x0": x0a[n], "y0": y0a[n], "x1": x1a[n], "y1": y1a[n],
                  "w0": lerp[n, 0:1], "w1": lerp[n, 1:2]}
            host_vals[col] = _host_column(ic, data32, W1a, b1a, W2a, b2a,
                                          W3a, b3v)
    return assemble(results, batch, xshift, (b3v, host_vals))


def kernel(z, data, W1, b1, W2, b2, W3, b3, x0, y0, x1, y1, lerp_weights,
           **_unused):
    in_maps, xshift = host_prep(data, W1, b1, W2, b2, W3, b3,
                                x0, y0, x1, y1, lerp_weights)
    res = run_device(in_maps)
    batch = np.asarray(z).shape[0]
    return postprocess(res.results, batch, xshift, data, W1, b1, W2, b2,
                       W3, b3, x0, y0, x1, y1, lerp_weights)
